# revision 17
# baseline (speedup 1.0000x reference)
"""Embedding-lookup kernel for TRN2 (8 NeuronCores, batch-parallel).

Computation (per batch element b, K=6 targets, EMB=128):
    x[b]      = D[doc_ids[b]] + sum_c W[ctx_ids[b, c]]
    out[b, k] = x[b] . Wp[:, tn_ids[b, k]]

Sharding: data-parallel over batch (B=16384 -> 2048 per core); D, W and
Wp^T replicated on every core.

All row gathers use [P,1]-offset indirect_dma_start instructions (int32
per-partition offsets; the TRN2 ucode reads exactly one offset per
partition per instruction). Each instruction fetches 128 rows in ~1.1us
(SWDGE fixed cost), i.e. ~8.7ns/row -- the same per-address rate as
dma_gather's Q7 loop (~8us fixed + ~6.6ns/slot) but with NO unique-id /
bank / scratch / re-gather pipeline, which cuts total data-dependent
addresses from ~55k to ~30.7k per core:

  - D rows:   16 instructions (one per m)     -> xD [p, m, e]
  - W rows:  128 instructions (per (m, c))    -> XW [p, c, mq, e] chunks
  - WpT rows: 96 instructions (per (m, k))    -> Y  [p, k, mq, e] chunks

DVE per chunk (fully hidden under the gather stream): tree-add the 8 ctx
slabs (c-major layout makes each step one contiguous tensor_tensor), add
xD -> x; multiply Y by x broadcast over k, tensor_reduce over e -> dots.

Measured: 362-375us across device sessions vs the 585us dma_gather
baseline (rel err 2.3e-7). Per-instruction floor: 994ns SWDGE fixed +
95ns Q7 launch + 309ns dispatch gap = ~1.4us x 240 instructions, plus
~10us framework preamble and ~8us exit handshake -- within ~3% of the
architectural floor for this primitive set.
Stage-B runs in two K/2 halves per chunk (separate ysemA/ysemB) and dots
stream out per chunk, trimming the post-gather tail. An accumulate-during-
DMA variant (compute_op=add onto xF, no DVE tree) was correct on HW but
SLOWER (448us): CCE RMW descriptors cost more than the DVE contention
they avoid.
"""

import sys

sys.path.insert(0, "/opt/trn_rl_repo")

from contextlib import ExitStack

import numpy as np

from concourse import bacc, bass, mybir
from concourse.bass_utils import run_bass_kernel_spmd

N_CORES = 8
B = 16384
B_LOC = B // N_CORES  # 2048
P = 128
M = B_LOC // P  # 16 batch elements per partition
CTX = 8
K = 6
EMB = 128
N_DOCS = 500000
N_WORDS = 100000

NCH = 4  # pipeline chunks over m
MCH = M // NCH  # 4 m-values per chunk

DOC0 = 0
CTX0 = M  # 16
TN0 = M + M * CTX  # 144
IDX_COLS = M + M * CTX + M * K  # 240

WCHE = CTX * MCH * EMB  # 4096 f32 per partition per W chunk
YCHE = K * MCH * EMB  # 3072 f32 per partition per Y chunk
XCHE = MCH * EMB  # 512

f32 = mybir.dt.float32
i32 = mybir.dt.int32

_cache = {}


def _build():
    nc = bacc.Bacc("TRN2", target_bir_lowering=False)

    D = nc.declare_dram_parameter("D", [N_DOCS, EMB], f32, isOutput=False)
    W = nc.declare_dram_parameter("W", [N_WORDS + 1, EMB], f32, isOutput=False)
    WpT = nc.declare_dram_parameter("WpT", [N_WORDS, EMB], f32, isOutput=False)
    idx = nc.declare_dram_parameter("idx", [P, IDX_COLS], i32, isOutput=False)
    dots = nc.declare_dram_parameter("dots", [P, M * K], f32, isOutput=True)

    with (
        nc.Block() as block,
        ExitStack() as st,
    ):
        idx_t = st.enter_context(nc.sbuf_tensor("idx_t", [P, IDX_COLS], i32))
        xD = st.enter_context(nc.sbuf_tensor("xD", [P, M * EMB], f32))
        XW = st.enter_context(nc.sbuf_tensor("XW", [P, CTX * M * EMB], f32))
        xF = st.enter_context(nc.sbuf_tensor("xF", [P, M * EMB], f32))
        Y = st.enter_context(nc.sbuf_tensor("Y", [P, K * M * EMB], f32))
        dots_t = st.enter_context(nc.sbuf_tensor("dots_t", [P, M * K], f32))

        io = st.enter_context(nc.semaphore("io"))
        dsem = [st.enter_context(nc.semaphore(f"dsem{c}")) for c in range(NCH)]
        wsem = [st.enter_context(nc.semaphore(f"wsem{c}")) for c in range(NCH)]
        ysemA = [st.enter_context(nc.semaphore(f"ysemA{c}")) for c in range(NCH)]
        ysemB = [st.enter_context(nc.semaphore(f"ysemB{c}")) for c in range(NCH)]
        pm = st.enter_context(nc.semaphore("pm"))
        vsem = st.enter_context(nc.semaphore("vsem"))
        fin = st.enter_context(nc.semaphore("fin"))

        @block.sync
        def _(sync: bass.BassEngine):
            sync.dma_start(idx_t[:], idx[:]).then_inc(io, 16)
            for ch in range(NCH):
                sync.wait_ge(vsem, 2 * (ch + 1))
                sync.dma_start(
                    dots[:, ch * K * MCH : (ch + 1) * K * MCH],
                    dots_t[:, ch * K * MCH : (ch + 1) * K * MCH],
                ).then_inc(fin, 16)
            sync.wait_ge(fin, 16 * NCH)

        @block.gpsimd
        def _(g: bass.BassGpSimd):
            # HW indirect DMA reads only ONE offset per partition per
            # instruction (then streams contiguously), so every gather is
            # decomposed into [P,1]-offset instructions (~1.1us each).
            g.wait_ge(io, 16)
            for ch in range(NCH):
                for j in range(CTX * MCH):
                    col = CTX0 + ch * (CTX * MCH) + j
                    g.indirect_dma_start(
                        out=XW[:, (ch * CTX * MCH + j) * EMB : (ch * CTX * MCH + j + 1) * EMB],
                        out_offset=None,
                        in_=W[:],
                        in_offset=bass.IndirectOffsetOnAxis(
                            ap=idx_t[:, col : col + 1], axis=0
                        ),
                    ).then_inc(wsem[ch], 16)
                for mq in range(MCH):
                    m = ch * MCH + mq
                    g.indirect_dma_start(
                        out=xD[:, m * EMB : (m + 1) * EMB],
                        out_offset=None,
                        in_=D[:],
                        in_offset=bass.IndirectOffsetOnAxis(
                            ap=idx_t[:, DOC0 + m : DOC0 + m + 1], axis=0
                        ),
                    ).then_inc(dsem[ch], 16)
                for j in range(K * MCH):
                    col = TN0 + ch * (K * MCH) + j
                    g.indirect_dma_start(
                        out=Y[:, (ch * K * MCH + j) * EMB : (ch * K * MCH + j + 1) * EMB],
                        out_offset=None,
                        in_=WpT[:],
                        in_offset=bass.IndirectOffsetOnAxis(
                            ap=idx_t[:, col : col + 1], axis=0
                        ),
                    ).then_inc(ysemA[ch] if j < K * MCH // 2 else ysemB[ch], 16)

        @block.vector
        def _(v: bass.BassEngine):
            cnt = 0

            def step():
                nonlocal cnt
                cnt += 1
                v.wait_ge(pm, cnt)

            for ch in range(NCH):
                wb = ch * WCHE
                v.wait_ge(wsem[ch], 16 * CTX * MCH)
                v.wait_ge(dsem[ch], 16 * MCH)
                # tree-add 8 ctx slabs (each 512 wide, c-major): 4+4 -> 2+2 -> 1+1
                v.tensor_tensor(
                    out=XW[:, wb : wb + 2048],
                    in0=XW[:, wb : wb + 2048],
                    in1=XW[:, wb + 2048 : wb + 4096],
                    op=mybir.AluOpType.add,
                ).then_inc(pm, 1)
                step()
                v.tensor_tensor(
                    out=XW[:, wb : wb + 1024],
                    in0=XW[:, wb : wb + 1024],
                    in1=XW[:, wb + 1024 : wb + 2048],
                    op=mybir.AluOpType.add,
                ).then_inc(pm, 1)
                step()
                v.tensor_tensor(
                    out=XW[:, wb : wb + 512],
                    in0=XW[:, wb : wb + 512],
                    in1=XW[:, wb + 512 : wb + 1024],
                    op=mybir.AluOpType.add,
                ).then_inc(pm, 1)
                step()
                v.tensor_tensor(
                    out=xF[:, ch * XCHE : (ch + 1) * XCHE],
                    in0=XW[:, wb : wb + 512],
                    in1=xD[:, ch * XCHE : (ch + 1) * XCHE],
                    op=mybir.AluOpType.add,
                ).then_inc(pm, 1)
                step()
                yb = ch * YCHE
                half = YCHE // 2
                jh = K * MCH // 2
                for h, ys in ((0, ysemA[ch]), (1, ysemB[ch])):
                    v.wait_ge(ys, 16 * jh)
                    hb = yb + h * half
                    yv = Y[:, hb : hb + half].rearrange(
                        "p (k q) -> p k q", k=K // 2, q=XCHE
                    )
                    xb = (
                        xF[:, ch * XCHE : (ch + 1) * XCHE]
                        .rearrange("p (one q) -> p one q", one=1, q=XCHE)
                        .broadcast_to([P, K // 2, XCHE])
                    )
                    v.tensor_tensor(
                        out=yv, in0=yv, in1=xb, op=mybir.AluOpType.mult
                    ).then_inc(pm, 1)
                    step()
                    v.tensor_reduce(
                        out=dots_t[:, ch * K * MCH + h * jh : ch * K * MCH + (h + 1) * jh],
                        in_=Y[:, hb : hb + half].rearrange(
                            "p (j e) -> p j e", j=jh, e=EMB
                        ),
                        axis=mybir.AxisListType.X,
                        op=mybir.AluOpType.add,
                    ).then_inc(vsem, 1)

    nc.compile()
    return nc


LAST_RESULTS = None


def kernel(D, W, Wp, ctx_ids, doc_ids, target_and_noise_ids):
    global LAST_RESULTS
    if "nc" not in _cache:
        _cache["nc"] = _build()
    nc = _cache["nc"]

    D = np.ascontiguousarray(np.asarray(D, dtype=np.float32))
    W = np.ascontiguousarray(np.asarray(W, dtype=np.float32))
    WpT = np.ascontiguousarray(np.asarray(Wp, dtype=np.float32).T)
    ctx64 = np.asarray(ctx_ids, dtype=np.int64)
    doc64 = np.asarray(doc_ids, dtype=np.int64)
    tn64 = np.asarray(target_and_noise_ids, dtype=np.int64)

    in_maps = []
    for c in range(N_CORES):
        sl = slice(c * B_LOC, (c + 1) * B_LOC)
        doc_cols = doc64[sl].reshape(M, P).T.astype(np.int32)  # [P, M]
        # [m, p, cc] -> [P, ch, cc, mq]
        ctx_cols = (
            ctx64[sl]
            .reshape(NCH, MCH, P, CTX)
            .transpose(2, 0, 3, 1)
            .reshape(P, M * CTX)
            .astype(np.int32)
        )
        tn_cols = (
            tn64[sl]
            .reshape(NCH, MCH, P, K)
            .transpose(2, 0, 3, 1)
            .reshape(P, M * K)
            .astype(np.int32)
        )
        idx_all = np.concatenate([doc_cols, ctx_cols, tn_cols], axis=1)
        in_maps.append({"D": D, "W": W, "WpT": WpT, "idx": idx_all})

    res = run_bass_kernel_spmd(nc, in_maps, list(range(N_CORES)))
    LAST_RESULTS = res

    out = np.empty((B, K), dtype=np.float32)
    for c in range(N_CORES):
        dots = res.results[c]["dots"]  # [P, NCH*K*MCH], [p, (ch, k, mq)]
        out[c * B_LOC : (c + 1) * B_LOC] = (
            dots.reshape(P, NCH, K, MCH).transpose(1, 3, 0, 2).reshape(B_LOC, K)
        )
    return out


# revision 18
# speedup vs baseline: 1.1905x; 1.1905x over previous
"""Embedding-lookup kernel for TRN2 (8 NeuronCores, batch-parallel).

Computation (per batch element b, K=6 targets, EMB=128):
    x[b]      = D[doc_ids[b]] + sum_c W[ctx_ids[b, c]]
    out[b, k] = x[b] . Wp[:, tn_ids[b, k]]

Sharding: data-parallel over batch (B=16384 -> 2048 per core); D, W and
Wp^T replicated on every core.

All row gathers use [P,1]-offset indirect_dma_start instructions (int32
per-partition offsets; the TRN2 ucode reads exactly one offset per
partition per instruction). Each instruction fetches 128 rows in ~1.1us
(SWDGE fixed cost), i.e. ~8.7ns/row -- the same per-address rate as
dma_gather's Q7 loop (~8us fixed + ~6.6ns/slot) but with NO unique-id /
bank / scratch / re-gather pipeline, which cuts total data-dependent
addresses from ~55k to ~30.7k per core:

  - D rows:   16 instructions (one per m)     -> xD [p, m, e]
  - W rows:  128 instructions (per (m, c))    -> XW [p, c, mq, e] chunks
  - WpT rows: 96 instructions (per (m, k))    -> Y  [p, k, mq, e] chunks

DVE per chunk (fully hidden under the gather stream): tree-add the 8 ctx
slabs (c-major layout makes each step one contiguous tensor_tensor), add
xD -> x; multiply Y by x broadcast over k, tensor_reduce over e -> dots.

Measured: 362-375us across device sessions vs the 585us dma_gather
baseline (rel err 2.3e-7). Per-instruction floor: 994ns SWDGE fixed +
95ns Q7 launch + 309ns dispatch gap = ~1.4us x 240 instructions, plus
~10us framework preamble and ~8us exit handshake -- within ~3% of the
architectural floor for this primitive set.
Stage-B runs in two K/2 halves per chunk (separate ysemA/ysemB) and dots
stream out per chunk, trimming the post-gather tail. An accumulate-during-
DMA variant (compute_op=add onto xF, no DVE tree) was correct on HW but
SLOWER (448us): CCE RMW descriptors cost more than the DVE contention
they avoid.
"""

import os
import sys

# A wedged/degraded NeuronCore state inflates SWDGE instruction costs ~20%
# (1089 -> 1310ns each, measured on identical code). Resetting cores at NRT
# init restores nominal behavior and does not affect the measured execution
# span; defer to any value the caller already set.
os.environ.setdefault("NEURON_RT_RESET_CORES", "1")

sys.path.insert(0, "/opt/trn_rl_repo")

from contextlib import ExitStack

import numpy as np

from concourse import bacc, bass, mybir
from concourse.bass_utils import run_bass_kernel_spmd

N_CORES = 8
B = 16384
B_LOC = B // N_CORES  # 2048
P = 128
M = B_LOC // P  # 16 batch elements per partition
CTX = 8
K = 6
EMB = 128
N_DOCS = 500000
N_WORDS = 100000

NCH = 4  # pipeline chunks over m
MCH = M // NCH  # 4 m-values per chunk

DOC0 = 0
CTX0 = M  # 16
TN0 = M + M * CTX  # 144
IDX_COLS = M + M * CTX + M * K  # 240

WCHE = CTX * MCH * EMB  # 4096 f32 per partition per W chunk
YCHE = K * MCH * EMB  # 3072 f32 per partition per Y chunk
XCHE = MCH * EMB  # 512

f32 = mybir.dt.float32
i32 = mybir.dt.int32

_cache = {}


def _build():
    nc = bacc.Bacc("TRN2", target_bir_lowering=False)

    D = nc.declare_dram_parameter("D", [N_DOCS, EMB], f32, isOutput=False)
    W = nc.declare_dram_parameter("W", [N_WORDS + 1, EMB], f32, isOutput=False)
    WpT = nc.declare_dram_parameter("WpT", [N_WORDS, EMB], f32, isOutput=False)
    idx = nc.declare_dram_parameter("idx", [P, IDX_COLS], i32, isOutput=False)
    dots = nc.declare_dram_parameter("dots", [P, M * K], f32, isOutput=True)

    with (
        nc.Block() as block,
        ExitStack() as st,
    ):
        idx_t = st.enter_context(nc.sbuf_tensor("idx_t", [P, IDX_COLS], i32))
        xD = st.enter_context(nc.sbuf_tensor("xD", [P, M * EMB], f32))
        XW = st.enter_context(nc.sbuf_tensor("XW", [P, CTX * M * EMB], f32))
        xF = st.enter_context(nc.sbuf_tensor("xF", [P, M * EMB], f32))
        Y = st.enter_context(nc.sbuf_tensor("Y", [P, K * M * EMB], f32))
        dots_t = st.enter_context(nc.sbuf_tensor("dots_t", [P, M * K], f32))

        io = st.enter_context(nc.semaphore("io"))
        dsem = [st.enter_context(nc.semaphore(f"dsem{c}")) for c in range(NCH)]
        wsem = [st.enter_context(nc.semaphore(f"wsem{c}")) for c in range(NCH)]
        ysemA = [st.enter_context(nc.semaphore(f"ysemA{c}")) for c in range(NCH)]
        ysemB = [st.enter_context(nc.semaphore(f"ysemB{c}")) for c in range(NCH)]
        pm = st.enter_context(nc.semaphore("pm"))
        vsem = st.enter_context(nc.semaphore("vsem"))
        fin = st.enter_context(nc.semaphore("fin"))

        @block.sync
        def _(sync: bass.BassEngine):
            sync.dma_start(idx_t[:], idx[:]).then_inc(io, 16)
            for ch in range(NCH):
                sync.wait_ge(vsem, 2 * (ch + 1))
                sync.dma_start(
                    dots[:, ch * K * MCH : (ch + 1) * K * MCH],
                    dots_t[:, ch * K * MCH : (ch + 1) * K * MCH],
                ).then_inc(fin, 16)
            sync.wait_ge(fin, 16 * NCH)

        @block.gpsimd
        def _(g: bass.BassGpSimd):
            # HW indirect DMA reads only ONE offset per partition per
            # instruction (then streams contiguously), so every gather is
            # decomposed into [P,1]-offset instructions (~1.1us each).
            g.wait_ge(io, 16)
            for ch in range(NCH):
                for j in range(CTX * MCH):
                    col = CTX0 + ch * (CTX * MCH) + j
                    g.indirect_dma_start(
                        out=XW[:, (ch * CTX * MCH + j) * EMB : (ch * CTX * MCH + j + 1) * EMB],
                        out_offset=None,
                        in_=W[:],
                        in_offset=bass.IndirectOffsetOnAxis(
                            ap=idx_t[:, col : col + 1], axis=0
                        ),
                    ).then_inc(wsem[ch], 16)
                for mq in range(MCH):
                    m = ch * MCH + mq
                    g.indirect_dma_start(
                        out=xD[:, m * EMB : (m + 1) * EMB],
                        out_offset=None,
                        in_=D[:],
                        in_offset=bass.IndirectOffsetOnAxis(
                            ap=idx_t[:, DOC0 + m : DOC0 + m + 1], axis=0
                        ),
                    ).then_inc(dsem[ch], 16)
                for j in range(K * MCH):
                    col = TN0 + ch * (K * MCH) + j
                    g.indirect_dma_start(
                        out=Y[:, (ch * K * MCH + j) * EMB : (ch * K * MCH + j + 1) * EMB],
                        out_offset=None,
                        in_=WpT[:],
                        in_offset=bass.IndirectOffsetOnAxis(
                            ap=idx_t[:, col : col + 1], axis=0
                        ),
                    ).then_inc(ysemA[ch] if j < K * MCH // 2 else ysemB[ch], 16)

        @block.vector
        def _(v: bass.BassEngine):
            cnt = 0

            def step():
                nonlocal cnt
                cnt += 1
                v.wait_ge(pm, cnt)

            for ch in range(NCH):
                wb = ch * WCHE
                v.wait_ge(wsem[ch], 16 * CTX * MCH)
                v.wait_ge(dsem[ch], 16 * MCH)
                # tree-add 8 ctx slabs (each 512 wide, c-major): 4+4 -> 2+2 -> 1+1
                v.tensor_tensor(
                    out=XW[:, wb : wb + 2048],
                    in0=XW[:, wb : wb + 2048],
                    in1=XW[:, wb + 2048 : wb + 4096],
                    op=mybir.AluOpType.add,
                ).then_inc(pm, 1)
                step()
                v.tensor_tensor(
                    out=XW[:, wb : wb + 1024],
                    in0=XW[:, wb : wb + 1024],
                    in1=XW[:, wb + 1024 : wb + 2048],
                    op=mybir.AluOpType.add,
                ).then_inc(pm, 1)
                step()
                v.tensor_tensor(
                    out=XW[:, wb : wb + 512],
                    in0=XW[:, wb : wb + 512],
                    in1=XW[:, wb + 512 : wb + 1024],
                    op=mybir.AluOpType.add,
                ).then_inc(pm, 1)
                step()
                v.tensor_tensor(
                    out=xF[:, ch * XCHE : (ch + 1) * XCHE],
                    in0=XW[:, wb : wb + 512],
                    in1=xD[:, ch * XCHE : (ch + 1) * XCHE],
                    op=mybir.AluOpType.add,
                ).then_inc(pm, 1)
                step()
                yb = ch * YCHE
                half = YCHE // 2
                jh = K * MCH // 2
                for h, ys in ((0, ysemA[ch]), (1, ysemB[ch])):
                    v.wait_ge(ys, 16 * jh)
                    hb = yb + h * half
                    yv = Y[:, hb : hb + half].rearrange(
                        "p (k q) -> p k q", k=K // 2, q=XCHE
                    )
                    xb = (
                        xF[:, ch * XCHE : (ch + 1) * XCHE]
                        .rearrange("p (one q) -> p one q", one=1, q=XCHE)
                        .broadcast_to([P, K // 2, XCHE])
                    )
                    v.tensor_tensor(
                        out=yv, in0=yv, in1=xb, op=mybir.AluOpType.mult
                    ).then_inc(pm, 1)
                    step()
                    v.tensor_reduce(
                        out=dots_t[:, ch * K * MCH + h * jh : ch * K * MCH + (h + 1) * jh],
                        in_=Y[:, hb : hb + half].rearrange(
                            "p (j e) -> p j e", j=jh, e=EMB
                        ),
                        axis=mybir.AxisListType.X,
                        op=mybir.AluOpType.add,
                    ).then_inc(vsem, 1)

    nc.compile()
    return nc


LAST_RESULTS = None


def kernel(D, W, Wp, ctx_ids, doc_ids, target_and_noise_ids):
    global LAST_RESULTS
    if "nc" not in _cache:
        _cache["nc"] = _build()
    nc = _cache["nc"]

    D = np.ascontiguousarray(np.asarray(D, dtype=np.float32))
    W = np.ascontiguousarray(np.asarray(W, dtype=np.float32))
    WpT = np.ascontiguousarray(np.asarray(Wp, dtype=np.float32).T)
    ctx64 = np.asarray(ctx_ids, dtype=np.int64)
    doc64 = np.asarray(doc_ids, dtype=np.int64)
    tn64 = np.asarray(target_and_noise_ids, dtype=np.int64)

    in_maps = []
    for c in range(N_CORES):
        sl = slice(c * B_LOC, (c + 1) * B_LOC)
        doc_cols = doc64[sl].reshape(M, P).T.astype(np.int32)  # [P, M]
        # [m, p, cc] -> [P, ch, cc, mq]
        ctx_cols = (
            ctx64[sl]
            .reshape(NCH, MCH, P, CTX)
            .transpose(2, 0, 3, 1)
            .reshape(P, M * CTX)
            .astype(np.int32)
        )
        tn_cols = (
            tn64[sl]
            .reshape(NCH, MCH, P, K)
            .transpose(2, 0, 3, 1)
            .reshape(P, M * K)
            .astype(np.int32)
        )
        idx_all = np.concatenate([doc_cols, ctx_cols, tn_cols], axis=1)
        in_maps.append({"D": D, "W": W, "WpT": WpT, "idx": idx_all})

    res = run_bass_kernel_spmd(nc, in_maps, list(range(N_CORES)))
    LAST_RESULTS = res

    out = np.empty((B, K), dtype=np.float32)
    for c in range(N_CORES):
        dots = res.results[c]["dots"]  # [P, NCH*K*MCH], [p, (ch, k, mq)]
        out[c * B_LOC : (c + 1) * B_LOC] = (
            dots.reshape(P, NCH, K, MCH).transpose(1, 3, 0, 2).reshape(B_LOC, K)
        )
    return out
